# revision 53
# baseline (speedup 1.0000x reference)
"""AFormer self-attention (relative_key_query position bias) on 8 trn2 NeuronCores.

Sharding: 24 (batch, head) pairs -> 8 cores x 3 heads, no collectives.
Core c handles batch c//4, heads 3*(c%4) .. 3*(c%4)+2.

v7 (v6 + tile_position packing: PE-array row/col concurrency):
  - q/k/v + eT/epT/bqkv duplicated across partition halves (dup via one
    SBUF->SBUF DMA per tensor per head; constants host-side)
  - K=64 matmuls (QK, T1/T2 band chunks, v_aug transposes) issued as
    adjacent pairs on partition halves 0-63 / 64-127 -> disjoint PE
    row-groups run concurrently (~2x effective throughput)
  - T1 merge transposes split into two K=64 halves (rows 0-63 -> out cols
    0-63, rows 64-127 -> cols 64-127) -> concurrent pair per 128-block
  - projection chains col-packed in pairs: chain n -> psum partitions 0-63,
    chain n+1 -> 64-127 (disjoint PE col-groups, concurrent)
  - otherwise identical to v6: transposed-scores S^T[r,l], flat cross-head
    software pipeline, 4-buf PSUM ring with rotating ACT/DVE evictors,
    diag-extract DMAs, deferred ctx finish.

Scaling 1/sqrt(64) folded host-side into q-eviction scale (covers QK, T1)
and into E^T (covers T2).
"""
import os
import sys
import numpy as np

sys.path.insert(0, '/opt/trn_rl_repo')

import concourse.bass as bass
import concourse.tile as tile
import concourse.mybir as mybir
from concourse import bacc
from concourse.ap import AP
from concourse.tile_rust import add_dep_helper

F32 = mybir.dt.float32
BF16 = mybir.dt.bfloat16
Exp = mybir.ActivationFunctionType.Exp
Copy = mybir.ActivationFunctionType.Copy
Ident = mybir.ActivationFunctionType.Identity

B, S, DM, H, D = 2, 2048, 768, 12, 64
MAX_POS = 2048
N_CORES = 8
HPC = 3                      # heads per core

# tile_position packing toggles (bisection / fallback)
# NOTE: mid-group (start=False) matmuls with row-base 64 fault the device;
# row-base-64 ops are only legal as accumulation-group openers. So the
# T1-merge split (PACK_T1M) is NOT usable — left as a flag for reference.
PACK_PROJ = True             # col-pack projection chain pairs
PACK_QK = True               # row-pack QK chunk pairs
PACK_T1M = False             # UNUSABLE: see note above
PACK_BANDS = True            # row-pack T1/T2 band chunks by parity


def build_nc(s=S, dm=DM, hpc=HPC, reps=1):
    """Build the per-core Bacc graph. Parametrized for small-config testing."""
    nt = s // 128            # l/r tiles
    mt = dm // 128           # contraction tiles for projections
    w2 = s + 128             # T2 band width
    hw = s // 2              # half width (1024)
    w1 = hw + 128            # T1 half-band width (1152)
    nheads = hpc * reps
    npc = s // 512           # proj chains per tensor
    npp = npc // 2           # proj chain pairs per tensor

    nc = bacc.Bacc(detect_race_conditions=False)
    hT_e = nc.declare_dram_parameter("hT", [dm, s], BF16, isOutput=False)
    wq_e = nc.declare_dram_parameter("wq", [dm, 64 * hpc], BF16, isOutput=False)
    wk_e = nc.declare_dram_parameter("wk", [dm, 64 * hpc], BF16, isOutput=False)
    wv_e = nc.declare_dram_parameter("wv", [dm, 64 * hpc], BF16, isOutput=False)
    bqkv_e = nc.declare_dram_parameter("bqkv", [128, 3 * hpc], F32, isOutput=False)
    bqkva_e = nc.declare_dram_parameter("bqkva", [128, 3 * hpc], F32, isOutput=False)
    eT_e = nc.declare_dram_parameter("eT", [128, 2 * s], BF16, isOutput=False)    # E^T/8 dup
    epT_e = nc.declare_dram_parameter("epT", [128, 2 * s], BF16, isOutput=False)  # E'^T dup
    mask_e = nc.declare_dram_parameter("maskT", [128, nt], F32, isOutput=False)
    eye_e = nc.declare_dram_parameter("eye", [128, 128], F32, isOutput=False)
    out_e = nc.declare_dram_parameter("out", [s, 64 * hpc], F32, isOutput=True)

    with tile.TileContext(nc) as tc:
        tc.race_detector_enabled = False
        with (
            tc.tile_pool(name="const", bufs=1) as cpool,
            tc.tile_pool(name="head", bufs=2) as hpool,
            tc.tile_pool(name="ctxf", bufs=1) as cfpool,
            tc.tile_pool(name="t1a", bufs=nt + 1) as apool,
            tc.tile_pool(name="t1b", bufs=nt + 1) as bpool1,
            tc.tile_pool(name="qband", bufs=3) as qbpool,
            tc.tile_pool(name="band", bufs=3) as bpool,
            tc.tile_pool(name="tt", bufs=3) as tpool,
            tc.tile_pool(name="prb", bufs=3) as ppool,
            tc.tile_pool(name="outp", bufs=4) as opool,
            tc.tile_pool(name="pw", bufs=4, space="PSUM") as pwork,
            tc.tile_pool(name="pctx", bufs=1, space="PSUM") as pctx,
        ):
            # ---- P0: load constants (host pre-casts to bf16, dups halves) ----
            # Order: per-m (wq[m], hTb[m]) so the first projection chain's
            # m-loop can start as soon as the first ~0.6MB lands, instead of
            # waiting out the full ~6.6MB load (~15us at HBM bw).
            hTb = []
            wqb, wkb, wvb = [], [], []
            for m in range(mt):
                t = cpool.tile([128, 64 * hpc], BF16, tag=f"wq{m}",
                               name=f"wq{m}")
                nc.scalar.dma_start(t[:], wq_e[128 * m:128 * (m + 1), :])
                wqb.append(t)
                t = cpool.tile([128, s], BF16, tag=f"hTb{m}", name=f"hTb{m}")
                eng = nc.sync if m % 2 == 0 else nc.scalar
                eng.dma_start(t[:], hT_e[128 * m:128 * (m + 1), :])
                hTb.append(t)
            for w_e, lst, nm in ((wk_e, wkb, "k"), (wv_e, wvb, "v")):
                for m in range(mt):
                    t = cpool.tile([128, 64 * hpc], BF16, tag=f"w{nm}{m}",
                                   name=f"w{nm}{m}")
                    nc.scalar.dma_start(t[:], w_e[128 * m:128 * (m + 1), :])
                    lst.append(t)
            eTb = cpool.tile([128, 2 * s], BF16, tag="eTb")
            epTb = cpool.tile([128, 2 * s], BF16, tag="epTb")
            nc.scalar.dma_start(eTb[:], eT_e[:, :])
            nc.sync.dma_start(epTb[:], epT_e[:, :])
            bqkv = cpool.tile([128, 3 * hpc], F32, tag="bqkv")
            nc.sync.dma_start(bqkv[:], bqkv_e[:, :])
            bqkva = cpool.tile([128, 3 * hpc], F32, tag="bqkva")
            nc.sync.dma_start(bqkva[:], bqkva_e[:, :])
            maskT = cpool.tile([128, nt], F32, tag="maskT")
            nc.sync.dma_start(maskT[:], mask_e[:, :])
            eye = cpool.tile([128, 128], F32, tag="eye")
            nc.sync.dma_start(eye[:], eye_e[:, :])
            eyeb = cpool.tile([128, 128], BF16, tag="eyeb")
            nc.vector.tensor_copy(eyeb[:], eye[:])

            # rotating evictor for PSUM 512-chunks (Pool cannot read PSUM).
            # ACT also owns the Exp evictions; ~3/7 ACT share measured best.
            ev_rot = [nc.scalar, nc.vector, nc.vector, nc.scalar, nc.vector,
                      nc.vector, nc.scalar]
            ev_i = [0]

            def evict(dst_ap, src_ap):
                eng = ev_rot[ev_i[0] % len(ev_rot)]
                ev_i[0] += 1
                if eng is nc.scalar:
                    return eng.activation(dst_ap, src_ap, Copy)
                return eng.tensor_copy(dst_ap, src_ap)

            tail_rot = [nc.vector, nc.scalar]
            tail_i = [0]

            def evict_tail(dst_ap, src_ap):
                eng = tail_rot[tail_i[0] % len(tail_rot)]
                tail_i[0] += 1
                if eng is nc.scalar:
                    return eng.activation(dst_ap, src_ap, Copy)
                return eng.tensor_copy(dst_ap, src_ap)

            # --------- per-head state ---------
            class Head:
                pass

            heads = []
            for i in range(nheads):
                hd = Head()
                hd.h = i % hpc
                hd.TT1A = [None] * nt
                hd.TT1B = [None] * nt
                hd.built = 0   # count of emitted proj chain PAIRS (q:0-1 k:2-3 v:4-5)
                hd.vaug_done = 0
                heads.append(hd)

            qb_prev_extract = [None, None, None]   # qband pool WAR (bufs=3)
            qb_i = [0]
            prev_t2_extract = [None, None, None]
            pending_finish = []
            pj_rot = [0]
            xr_i = [0]           # extract-DMA ring round-robin (sync/scalar)

            def proj_pair(hd, t):
                """Emit projection chain pair t (0..3*npp-1): col-packed."""
                if t == 0:
                    hd.qTb = hpool.tile([128, s], BF16, tag="qTb", name="qTb")
                    hd.kTb = hpool.tile([128, s], BF16, tag="kTb", name="kTb")
                    hd.vTb = hpool.tile([64, s], BF16, tag="vTb", name="vTb")
                    hd.v_aug = hpool.tile([128, nt, 65], BF16, tag="v_aug",
                                          name="v_aug")
                wb, dst, scale, bcol = (
                    (wqb, hd.qTb, 0.125, hd.h),
                    (wkb, hd.kTb, 1.0, hpc + hd.h),
                    (wvb, hd.vTb, 1.0, 2 * hpc + hd.h),
                )[t // npp]
                p = t % npp
                n0, n1 = 2 * p, 2 * p + 1
                hc = slice(64 * hd.h, 64 * hd.h + 64)
                if PACK_PROJ:
                    ps = pwork.tile([128, 512], F32, tag="pw", name="pspj")
                    for m in range(mt):
                        nc.tensor.matmul(ps[0:64, :512], wb[m][:, hc],
                                         hTb[m][:, 512 * n0:512 * (n0 + 1)],
                                         start=(m == 0), stop=(m == mt - 1),
                                         skip_group_check=True)
                        nc.tensor.matmul(ps[64:128, :512], wb[m][:, hc],
                                         hTb[m][:, 512 * n1:512 * (n1 + 1)],
                                         start=(m == 0), stop=(m == mt - 1),
                                         skip_group_check=True)
                    for n, lo, pslc in ((n0, 0, ps[0:64, :512]),
                                        (n1, 64, ps[64:128, :512])):
                        dcols = slice(512 * n, 512 * (n + 1))
                        if pj_rot[0] % 2 == 0:
                            nc.vector.tensor_scalar(
                                dst[0:64, dcols], pslc,
                                bqkv[lo:lo + 64, bcol:bcol + 1], scale,
                                mybir.AluOpType.add, mybir.AluOpType.mult)
                        else:
                            nc.scalar.activation(
                                dst[0:64, dcols], pslc, Ident,
                                bias=bqkva[lo:lo + 64, bcol:bcol + 1],
                                scale=scale)
                        pj_rot[0] += 1
                else:
                    for n in (n0, n1):
                        ps = pwork.tile([128, 512], F32, tag="pw", name="pspj")
                        for m in range(mt):
                            nc.tensor.matmul(ps[:64, :512], wb[m][:, hc],
                                             hTb[m][:, 512 * n:512 * (n + 1)],
                                             start=(m == 0), stop=(m == mt - 1))
                        nc.vector.tensor_scalar(
                            dst[0:64, 512 * n:512 * (n + 1)], ps[:64, :512],
                            bqkv[0:64, bcol:bcol + 1], scale,
                            mybir.AluOpType.add, mybir.AluOpType.mult)
                if p == npp - 1 and t // npp < 2:
                    # duplicate rows 0-63 -> 64-127 for row-packed consumers
                    # (q and k only; vTb is consumed unpacked by v_aug).
                    # SWDGE ring keeps the HWDGE rings free for extracts.
                    nc.gpsimd.dma_start(dst[64:128, :], dst[0:64, :])
                hd.built = t + 1

            def emit_vaug_pair(hd):
                c = hd.vaug_done
                if c >= nt:
                    return
                n = min(2, nt - c)
                hd.vaug_done += n
                ps = pwork.tile([128, 512], F32, tag="pw", name="psva")
                for j in range(n):
                    nc.tensor.matmul(ps[:, 64 * j:64 * (j + 1)],
                                     hd.vTb[0:64, 128 * (c + j):128 * (c + j + 1)],
                                     eyeb[0:64, 0:64],
                                     start=True, stop=True,
                                     skip_group_check=True)
                nc.vector.tensor_copy(hd.v_aug[:, c:c + n, 0:64], ps[:, :64 * n])
                nc.gpsimd.memset(hd.v_aug[:, c:c + n, 64:65], 1.0)

            def produce_t1_half(hd, c, half):
                """One T1 half-band for l-tile c -> TT1A/TT1B [128, hw].
                Chunks alternate partition halves -> PE row-group pairs."""
                QB = qbpool.tile([128, w1], BF16, tag="QB", name="QB")
                slot = qb_i[0] % 3
                qb_i[0] += 1
                jb = s - 128 * (c + 1) + (hw if half else 0)
                evs = []
                for idx, off in enumerate(range(0, w1, 512)):
                    cw = min(512, w1 - off)
                    rb_ = 64 * (idx % 2) if PACK_BANDS else 0
                    ps = pwork.tile([128, 512], F32, tag="pw", name="pst1")
                    nc.tensor.matmul(ps[:, :cw],
                                     hd.qTb[rb_:rb_ + 64, 128 * c:128 * (c + 1)],
                                     epTb[rb_:rb_ + 64, jb + off:jb + off + cw],
                                     start=True, stop=True)
                    if cw < 512:
                        ev = evict_tail(QB[:, off:off + cw], ps[:, :cw])
                    else:
                        ev = evict(QB[:, off:off + cw], ps[:, :cw])
                    if qb_prev_extract[slot] is not None:
                        add_dep_helper(ev.ins, qb_prev_extract[slot].ins,
                                       reason="QB WAR")
                    evs.append(ev)
                TT1 = (apool if half == 0 else bpool1).tile(
                    [128, hw], BF16, tag="TT1", name="TT1")
                src = AP(QB[:, :].tensor, 127, [[w1 - 1, 128], [1, hw]])
                xeng = nc.sync if xr_i[0] % 2 == 0 else nc.scalar
                xr_i[0] += 1
                i_t1 = xeng.dma_start(TT1[:, :], src)
                for ev in evs:
                    add_dep_helper(i_t1.ins, ev.ins, reason="T1 diag read")
                qb_prev_extract[slot] = i_t1
                if half == 0:
                    hd.TT1A[c] = TT1
                else:
                    hd.TT1B[c] = TT1

            def produce_band_chunks(hd, rt, lo, hi, state):
                if 'KEbb' not in state:
                    KEbb = bpool.tile([128, w2], BF16, tag="KEbb", name="KEbb")
                    state['KEbb'] = KEbb
                    state['evs'] = []
                KEbb = state['KEbb']
                jb2 = s - 128 * (rt + 1)
                nch = (w2 + 511) // 512
                for ci in range(lo, min(hi, nch)):
                    off = 512 * ci
                    cw = min(512, w2 - off)
                    rb_ = 64 * (ci % 2) if PACK_BANDS else 0
                    ps = pwork.tile([128, 512], F32, tag="pw", name="pst2")
                    nc.tensor.matmul(ps[:, :cw],
                                     hd.kTb[rb_:rb_ + 64, 128 * rt:128 * (rt + 1)],
                                     eTb[rb_:rb_ + 64, jb2 + off:jb2 + off + cw],
                                     start=True, stop=True)
                    if cw < 512:
                        ev = evict_tail(KEbb[:, off:off + cw], ps[:, :cw])
                    else:
                        ev = evict(KEbb[:, off:off + cw], ps[:, :cw])
                    if prev_t2_extract[rt % 3] is not None:
                        add_dep_helper(ev.ins, prev_t2_extract[rt % 3].ins,
                                       reason="KEbb WAR")
                    state['evs'].append(ev)

            def finish_band(rt, state):
                KEbb = state['KEbb']
                TT2 = tpool.tile([128, s], BF16, tag="TT2", name="TT2")
                src2 = AP(KEbb[:, :].tensor, 127, [[w2 - 1, 128], [1, s]])
                xeng = nc.sync if xr_i[0] % 2 == 0 else nc.scalar
                xr_i[0] += 1
                i_t2 = xeng.dma_start(TT2[:, :], src2)
                for ev in state['evs']:
                    add_dep_helper(i_t2.ins, ev.ins, reason="T2 diag read")
                prev_t2_extract[rt % 3] = i_t2
                return TT2

            def make_finish(hd, c, ctxTf):
                def fin():
                    ps = pwork.tile([128, 512], F32, tag="pw", name="psfin")
                    nc.tensor.matmul(ps[:, :65],
                                     ctxTf[:, 128 * c:128 * (c + 1)],
                                     eyeb[:65, :65], start=True, stop=True)
                    rec = opool.tile([128, 1], F32, tag="rec", name="rec")
                    nc.vector.reciprocal(rec[:], ps[:, 64:65])
                    ot = opool.tile([128, 64], F32, tag="ot", name="ot")
                    nc.vector.tensor_scalar_mul(ot[:], ps[:, 0:64], rec[:])
                    nc.gpsimd.dma_start(
                        out_e[128 * c:128 * (c + 1),
                              64 * hd.h:64 * (hd.h + 1)], ot[:])
                return fin

            def flush_finish(k):
                todo = pending_finish[:k]
                del pending_finish[:len(todo)]
                for fn in todo:
                    fn()

            # ---- prologue: head 0 projections + v_aug + its A halves ----
            hd0 = heads[0]
            a0 = [0]
            for t in range(3 * npp):
                proj_pair(hd0, t)
                if t >= npp:        # q complete (incl dup) -> A halves
                    for _ in range(2):
                        if a0[0] < nt:
                            produce_t1_half(hd0, a0[0], 0)
                            a0[0] += 1
            while a0[0] < nt:
                produce_t1_half(hd0, a0[0], 0)
                a0[0] += 1
            while hd0.vaug_done < nt:
                emit_vaug_pair(hd0)

            # ---- steady-state: one iteration per (head, rt) ----
            for hi in range(nheads):
                hd = heads[hi]
                nxt = heads[hi + 1] if hi + 1 < nheads else None
                ctxT = pctx.tile([65, s], F32, tag="ctxT", name="ctxT")
                band_q = {}

                for rt in range(nt):
                    bstate = {}
                    if rt == 0:
                        b0, b1 = {}, {}
                        produce_band_chunks(hd, 0, 0, 5, b0)
                        band_q[0] = finish_band(0, b0)
                        produce_band_chunks(hd, 1, 0, 5, b1)
                        band_q[1] = finish_band(1, b1)

                    # ---- static side-work for this rt ----
                    half = nt // 2
                    side = []
                    if rt < half:
                        side.append(lambda rt=rt: flush_finish(2))
                        side.append(lambda c=2 * rt: produce_t1_half(hd, c, 1))
                        side.append(lambda c=2 * rt + 1: produce_t1_half(hd, c, 1))
                        if rt >= half - 2 and nxt is not None:
                            t = rt - (half - 2)
                            if t < npp:
                                side.append(lambda t=t: proj_pair(nxt, t))
                    elif nxt is not None and nt >= 16:
                        if rt < half + 4:
                            side.append(lambda t=npp + (rt - half): proj_pair(nxt, t))
                            if rt >= half + 2:
                                for _ in range(2):
                                    side.append(lambda: emit_vaug_pair(nxt))
                        elif rt < half + 6:
                            for _ in range(2):
                                side.append(lambda: emit_vaug_pair(nxt))
                        side.append(lambda c=2 * (rt - half): produce_t1_half(nxt, c, 0))
                        side.append(lambda c=2 * (rt - half) + 1: produce_t1_half(nxt, c, 0))
                    elif nxt is not None and rt == half:
                        # small-config fallback: emit all next-head work now
                        for t in range(npp, 3 * npp):
                            side.append(lambda t=t: proj_pair(nxt, t))
                        for _ in range((nt + 1) // 2):
                            side.append(lambda: emit_vaug_pair(nxt))
                        for c in range(nt):
                            side.append(lambda c=c: produce_t1_half(nxt, c, 0))

                    def pop_side(k=1):
                        for _ in range(min(k, len(side))):
                            side.pop(0)()

                    # ---- interleaved emission for this rt ----
                    do_band = rt + 2 < nt
                    if do_band:
                        produce_band_chunks(hd, rt + 2, 0, 2, bstate)
                    pop_side()
                    if do_band:
                        produce_band_chunks(hd, rt + 2, 2, 4, bstate)
                    pop_side()
                    if do_band:
                        produce_band_chunks(hd, rt + 2, 4, 5, bstate)
                        band_q[rt + 2] = finish_band(rt + 2, bstate)
                    TT2 = band_q.pop(rt)

                    TT1s = hd.TT1A if rt < half else hd.TT1B
                    rb = 128 * (rt % half)

                    # ---- scores chunk pairs, PV lagging by one pair ----
                    PRB = ppool.tile([128, s], BF16, tag="PRB", name="PRB")

                    def emit_pv(n):
                        nc.tensor.matmul(ctxT[:, 512 * n:512 * (n + 1)],
                                         hd.v_aug[:, rt, :],
                                         PRB[:, 512 * n:512 * (n + 1)],
                                         start=(rt == 0), stop=(rt == nt - 1),
                                         skip_group_check=True)

                    def t1m_half(ps, c, col, hi):
                        b = 64 * hi
                        nc.tensor.matmul(ps[:, col + b:col + b + 64],
                                         TT1s[c][b:b + 64, rb:rb + 128],
                                         eyeb[b:b + 64, b:b + 64],
                                         start=False, stop=False,
                                         skip_group_check=True)

                    def t1m_full(ps, c, col):
                        nc.tensor.matmul(ps[:, col:col + 128],
                                         TT1s[c][:, rb:rb + 128], eyeb[:],
                                         start=False, stop=False,
                                         skip_group_check=True)

                    def t2m_exp(ps, n):
                        nc.tensor.matmul(ps[:, :512], eyeb[:],
                                         TT2[:, 512 * n:512 * (n + 1)],
                                         start=False, stop=True,
                                         skip_group_check=True)
                        nc.scalar.activation(PRB[:, 512 * n:512 * (n + 1)],
                                             ps[:, :512], Exp,
                                             bias=maskT[:, rt:rt + 1])

                    def merges_pair(ps0, n0, ps1, n1):
                        # interleave so concurrent row-group pairs always hit
                        # different PSUM banks: (ps0,lo)+(ps1,hi), (ps0,hi)+(ps1,lo)
                        for ci in range(4):
                            col = 128 * ci
                            c0, c1 = 4 * n0 + ci, 4 * n1 + ci
                            if PACK_T1M:
                                t1m_half(ps0, c0, col, 0)
                                t1m_half(ps1, c1, col, 1)
                                t1m_half(ps0, c0, col, 1)
                                t1m_half(ps1, c1, col, 0)
                            else:
                                t1m_full(ps0, c0, col)
                                t1m_full(ps1, c1, col)

                    npair = s // 1024
                    for p in range(npair):
                        n0, n1 = 2 * p, 2 * p + 1
                        qkb = 64 if PACK_QK else 0
                        ps0 = pwork.tile([128, 512], F32, tag="pw", name="psqk0")
                        ps1 = pwork.tile([128, 512], F32, tag="pw", name="psqk1")
                        nc.tensor.matmul(ps0[:, :512],
                                         hd.kTb[0:64, 128 * rt:128 * (rt + 1)],
                                         hd.qTb[0:64, 512 * n0:512 * (n0 + 1)],
                                         start=True, stop=False,
                                         skip_group_check=True)
                        nc.tensor.matmul(ps1[:, :512],
                                         hd.kTb[qkb:qkb + 64, 128 * rt:128 * (rt + 1)],
                                         hd.qTb[qkb:qkb + 64, 512 * n1:512 * (n1 + 1)],
                                         start=True, stop=False,
                                         skip_group_check=True)
                        merges_pair(ps0, n0, ps1, n1)
                        t2m_exp(ps0, n0)
                        if n0 > 0:
                            emit_pv(n0 - 2)
                        pop_side()
                        t2m_exp(ps1, n1)
                        if n1 > 2:
                            emit_pv(n1 - 2)
                        pop_side()
                    emit_pv(s // 512 - 2)
                    emit_pv(s // 512 - 1)
                    pop_side(len(side))

                # ---- deferred finish ----
                ctxTf = cfpool.tile([65, s], BF16, tag="ctxTf", name="ctxTf")
                # chunked so the last head's finish chain starts immediately
                for fc in range(0, s, 512):
                    if (fc // 512) % 2 == 0:
                        nc.scalar.activation(ctxTf[:, fc:fc + 512],
                                             ctxT[:, fc:fc + 512], Copy)
                    else:
                        nc.vector.tensor_copy(ctxTf[:, fc:fc + 512],
                                              ctxT[:, fc:fc + 512])
                for c in range(nt):
                    pending_finish.append(make_finish(hd, c, ctxTf))
                hd.TT1A = hd.TT1B = None

            flush_finish(len(pending_finish))

    nc.compile()
    return nc


_NC_CACHE = {}


def _get_nc(key=(S, DM, HPC)):
    if key not in _NC_CACHE:
        _NC_CACHE[key] = build_nc(*key)
    return _NC_CACHE[key]


def _bf16(a):
    import ml_dtypes
    return np.asarray(a, np.float32).astype(ml_dtypes.bfloat16)


def make_in_maps(hidden_states, attention_mask, Wq, bq, Wk, bk, Wv, bv, dist_emb,
                 s=S, hpc=HPC, n_cores=N_CORES):
    nt = s // 128
    e = np.asarray(dist_emb, np.float32)           # [2s-1, 64]
    eT = np.zeros((128, 2 * s), np.float32)
    eT[0:64, :2 * s - 1] = e.T * 0.125
    eT[64:128] = eT[0:64]
    epT = np.zeros((128, 2 * s), np.float32)
    epT[0:64, :2 * s - 1] = e[::-1].T
    epT[64:128] = epT[0:64]
    eT = _bf16(eT)
    epT = _bf16(epT)
    eye = np.eye(128, dtype=np.float32)
    groups_per_b = n_cores // np.asarray(hidden_states).shape[0]
    in_maps = []
    for c in range(n_cores):
        b = c // groups_per_b
        h0 = hpc * (c % groups_per_b)
        cols = slice(64 * h0, 64 * (h0 + hpc))
        bqkv = np.stack([np.asarray(v, np.float32)[cols].reshape(hpc, 64)
                         for v in (bq, bk, bv)], 0)  # [3, hpc, 64]
        bq64 = bqkv.reshape(3 * hpc, 64).T           # [64, 3*hpc]
        bq128 = np.concatenate([bq64, bq64], axis=0)  # [128, 3*hpc] dup
        in_maps.append({
            "hT": _bf16(np.asarray(hidden_states, np.float32)[b].T),
            "wq": _bf16(np.asarray(Wq, np.float32)[:, cols]),
            "wk": _bf16(np.asarray(Wk, np.float32)[:, cols]),
            "wv": _bf16(np.asarray(Wv, np.float32)[:, cols]),
            "bqkv": np.ascontiguousarray(bq128),
            "bqkva": np.ascontiguousarray(
                bq128 * np.repeat([0.125, 1.0, 1.0], hpc)[None, :]),
            "eT": eT, "epT": epT,
            "maskT": np.ascontiguousarray(
                np.asarray(attention_mask, np.float32)[b, 0, 0]
                .reshape(nt, 128).T),
            "eye": eye,
        })
    return in_maps


def _build_exec(nc):
    """AOT-compile the 8-core SPMD executable once; cache for reuse."""
    import jax
    from jax.sharding import Mesh, PartitionSpec, NamedSharding
    from jax.experimental.shard_map import shard_map
    from concourse import bass2jax
    from concourse.bass2jax import _bass_exec_p, install_neuronx_cc_hook

    install_neuronx_cc_hook()
    pname = nc.partition_id_tensor.name if nc.partition_id_tensor else None
    in_names, out_names, out_avals, zero_outs = [], [], [], []
    for alloc in nc.m.functions[0].allocations:
        if not isinstance(alloc, mybir.MemoryLocationSet):
            continue
        name = alloc.memorylocations[0].name
        if alloc.kind == "ExternalInput":
            if name != pname:
                in_names.append(name)
        elif alloc.kind == "ExternalOutput":
            shape = tuple(alloc.tensor_shape)
            dtype = mybir.dt.np(alloc.dtype)
            out_names.append(name)
            out_avals.append(jax.core.ShapedArray(shape, dtype))
            zero_outs.append(np.zeros(shape, dtype))
    n_in, n_out = len(in_names), len(out_avals)
    all_names = in_names + out_names + ([pname] if pname else [])

    def _body(*args):
        ops = list(args)
        if pname is not None:
            ops.append(bass2jax.partition_id_tensor())
        return tuple(_bass_exec_p.bind(
            *ops, out_avals=tuple(out_avals), in_names=tuple(all_names),
            out_names=tuple(out_names), lowering_input_output_aliases=(),
            sim_require_finite=True, sim_require_nnan=True, nc=nc))

    devices = jax.devices()[:N_CORES]
    mesh = Mesh(np.asarray(devices), ("core",))
    spec = PartitionSpec("core")
    jitted = jax.jit(
        shard_map(_body, mesh=mesh, in_specs=(spec,) * (n_in + n_out),
                  out_specs=(spec,) * n_out, check_rep=False),
        donate_argnums=tuple(range(n_in, n_in + n_out)), keep_unused=True)
    sh = NamedSharding(mesh, spec)

    class Exec:
        pass

    ex = Exec()
    ex.jitted, ex.sh = jitted, sh
    ex.in_names, ex.out_names, ex.zero_outs = in_names, out_names, zero_outs
    ex.out_shapes = [tuple(a.shape) for a in out_avals]
    return ex


_EXEC_CACHE = {}


def get_exec(key=(S, DM, HPC), reps=1):
    if (key, reps) not in _EXEC_CACHE:
        _EXEC_CACHE[(key, reps)] = _build_exec(build_nc(*key, reps))
    return _EXEC_CACHE[(key, reps)]


def run_exec(ex, in_maps):
    """Execute the cached SPMD executable on per-core input dicts."""
    import jax
    per_core = [[np.asarray(m[nm]) for nm in ex.in_names] for m in in_maps]
    concat_in = [np.concatenate([per_core[c][i] for c in range(N_CORES)], axis=0)
                 for i in range(len(ex.in_names))]
    dev_in = [jax.device_put(a, ex.sh) for a in concat_in]
    zeros = [jax.device_put(
        np.zeros((N_CORES * z.shape[0], *z.shape[1:]), z.dtype), ex.sh)
        for z in ex.zero_outs]
    outs = ex.jitted(*dev_in, *zeros)
    return [{nm: np.asarray(outs[i]).reshape(N_CORES, *ex.out_shapes[i])[c]
             for i, nm in enumerate(ex.out_names)}
            for c in range(N_CORES)]


def kernel(hidden_states, attention_mask, Wq, bq, Wk, bk, Wv, bv, dist_emb):
    in_maps = make_in_maps(hidden_states, attention_mask,
                           Wq, bq, Wk, bk, Wv, bv, dist_emb)
    try:
        results = run_exec(get_exec(), in_maps)
    except Exception:
        # fallback: stock per-call path
        from concourse.bass_utils import run_bass_kernel_spmd
        results = run_bass_kernel_spmd(
            _get_nc(), in_maps, list(range(N_CORES))).results
    out = np.empty((B, S, DM), np.float32)
    groups_per_b = N_CORES // B
    for c in range(N_CORES):
        b = c // groups_per_b
        h0 = HPC * (c % groups_per_b)
        out[b, :, 64 * h0:64 * (h0 + HPC)] = results[c]["out"]
    return out
